# revision 3
# baseline (speedup 1.0000x reference)
"""CavAttention Trainium2 kernel, v2.

Computation (per spatial location (b,h,w), L=5 "cav" slots, 8 heads x 32 dim):
  qkv = x @ w_qkv ; att = softmax_j(mask * q_i.k_j / sqrt(d)) ; o = att @ v ; out = o @ w_out + b_out

Distribution: shard the H axis (48) across the 8 cores (6 each); weights replicated.

Per-core strategy (bf16 compute, fp32 accumulation in PSUM):
  - locations ride the 128 SBUF partitions for projections/softmax/AV;
    q,k are produced transposed [(m,d), (l,p)] so the 5x5 QK contraction over
    d becomes narrow-output PE matmuls (band matrix sums d within heads).
  - softmax skips the max-subtraction (logits ~ N(0,1): exp is safe);
    the mask enters as -30*(1-mask) added to logits by one K=5 PE matmul
    whose lhsT is the mask row loaded transposed by the DMA.
  - AV products are one big DVE bf16 multiply; the sum over j is folded
    into PE identity-matmuls accumulating in PSUM, which directly yields
    the transposed operand for the output projection.
  - the tile loop is software-pipelined 4 deep so PE never waits in-order
    on a cross-engine producer of the same tile.
  - GPSIMD cannot touch PSUM: it only does SBUF->SBUF work (x cast, part
    of the QK products); all PSUM evacuations are on ACT/DVE.
"""

import numpy as np

B, L, H, W, C = 2, 5, 48, 176, 256
HEADS, DIM_HEAD = 8, 32
INNER = HEADS * DIM_HEAD  # 256
SCALE = DIM_HEAD ** -0.5
NBIG = 176.0  # mask logit offset (scaled by SCALE inside exp: 176/sqrt(32) ~ 31)
NCORES = 8
HP = H // NCORES  # 6 h-planes per core
NBH = B * HP      # 12 (b,h) blocks per core
LOCS = NBH * W    # 2112 locations per core
PTILE = 128
NTILES = (LOCS + PTILE - 1) // PTILE  # 17

_cached = {}


def _pieces(s, e):
    """Split flat loc range [s,e) into (p0, b, h, w0, w1) pieces within (b,h) blocks."""
    out = []
    cur = s
    while cur < e:
        bh = cur // W
        w0 = cur % W
        w1 = min(W, w0 + (e - cur))
        out.append((cur - s, bh // HP, bh % HP, w0, w1))
        cur += w1 - w0
    return out


def _host_consts():
    """Precomputed constant blob DMA'd once per core (bf16)."""
    import ml_dtypes

    bf = ml_dtypes.bfloat16
    # consts [128, 340]: ident | H4 | MJ (j-expansion over (i,j,m))
    consts = np.zeros((128, 340), dtype=np.float32)
    consts[:, :128] = np.eye(128, dtype=np.float32)
    for m in range(4):  # H4: band over d within a 4-head chunk
        consts[m * 32:(m + 1) * 32, 128 + m] = 1.0
    for jp in range(L):  # MJ[j', (i,j,m)] = delta_{j,j'}
        for i in range(L):
            for m in range(HEADS):
                consts[jp, 132 + (i * L + jp) * HEADS + m] = 1.0
    return consts.astype(bf)


def _host_weights(w_qkv, w_out, b_out):
    import ml_dtypes

    bf = ml_dtypes.bfloat16
    wq = w_qkv[:, 0:INNER]                      # [c, (m,d)]
    wk = w_qkv[:, INNER:2 * INNER]              # [c, (m,d)]
    wv = w_qkv[:, 2 * INNER:3 * INNER]          # [c, (m,d)]
    # v is consumed as [p, (l, d, m)]: reorder wv cols (m,d) -> (d,m)
    wv2 = wv.reshape(C, HEADS, DIM_HEAD).transpose(0, 2, 1).reshape(C, INNER)
    # out-proj contracts over (d,m): reorder w_out rows (m,d) -> (d,m)
    wo2 = w_out.reshape(HEADS, DIM_HEAD, C).transpose(1, 0, 2).reshape(INNER, C)
    wfused = np.concatenate([wq, wk, wv2, wo2], axis=1)  # [256, 1024]
    return wfused.astype(bf), b_out.reshape(1, C).astype(bf)


def _build_bass():
    import concourse.bass as bass
    import concourse.bacc as bacc
    import concourse.tile as tile
    from concourse import mybir

    f32 = mybir.dt.float32
    bf16 = mybir.dt.bfloat16
    i32 = mybir.dt.int32

    nc = bacc.Bacc()
    x_d = nc.dram_tensor("x", [B, L, HP, W, C], f32, kind="ExternalInput")
    m_d = nc.dram_tensor("mask", [B, HP, W, 1, L], i32, kind="ExternalInput")
    wf_d = nc.dram_tensor("wfused", [C, 4 * C], bf16, kind="ExternalInput")
    bout_d = nc.dram_tensor("bconst", [1, C], bf16, kind="ExternalInput")
    cst_d = nc.dram_tensor("consts", [128, 340], bf16, kind="ExternalInput")
    out_d = nc.dram_tensor("out", [B, L, HP, W, C], f32, kind="ExternalOutput")

    with tile.TileContext(nc) as tc:
        with (
            tc.tile_pool(name="singles", bufs=1) as singles,
            tc.tile_pool(name="work", bufs=3) as work,
            tc.tile_pool(name="ps_a", bufs=2, space="PSUM") as ps_a,  # 2-bank tiles
            tc.tile_pool(name="ps_b", bufs=2, space="PSUM") as ps_b,  # 1-bank tiles
            tc.tile_pool(name="ps_pa", bufs=1, space="PSUM") as ps_pa,  # logits
        ):
            # ---- one-time constants ----
            cst = singles.tile([128, 340], bf16)
            nc.sync.dma_start(out=cst, in_=cst_d[:, :])
            ident = cst[:, 0:128]
            h4 = cst[:, 128:132]
            mj = cst[0:L, 132:332]
            wf = singles.tile([128, 2, 4 * C], bf16)
            for cc in range(2):
                nc.sync.dma_start(out=wf[:, cc, :], in_=wf_d[cc * 128:(cc + 1) * 128, :])
            bconst = singles.tile([1, C], bf16)
            nc.sync.dma_start(out=bconst, in_=bout_d[:, :])
            ones1 = singles.tile([1, 128], bf16)
            nc.gpsimd.memset(ones1, 1.0)

            state = {}

            def stage_load(t):
                """DMA loads for tile t (prefetched one iteration ahead)."""
                s = t * PTILE
                e = min(s + PTILE, LOCS)
                P = e - s
                pieces = _pieces(s, e)

                x_raw = work.tile([128, L, C], f32, tag="x_raw", bufs=2)
                for (p0, b, h, w0, w1) in pieces:
                    nc.sync.dma_start(
                        out=x_raw[p0:p0 + (w1 - w0), :, :],
                        in_=x_d[b, :, h, w0:w1, :].transpose([1, 0, 2]),
                    )
                # mask loaded transposed: [L(j), p]
                maskiT = work.tile([L, 128], i32, tag="maskiT")
                for (p0, b, h, w0, w1) in pieces:
                    nc.sync.dma_start(
                        out=maskiT[:, p0:p0 + (w1 - w0)],
                        in_=m_d[b, h, w0:w1, 0, :].transpose([1, 0]),
                    )
                xb = work.tile([128, L, C], bf16, tag="xb")
                nc.gpsimd.tensor_copy(out=xb[:P], in_=x_raw[:P])
                state[t] = dict(P=P, pieces=pieces, xb=xb, maskiT=maskiT)

            def stage_proj(t):
                """Cast, transposes, projections for tile t."""
                st = state[t]
                P, xb = st["P"], st["xb"]

                # x^T via PE identity-matmuls: xt [(c)2ch, l, p] bf16
                xt = work.tile([128, 2, L, 128], bf16, tag="xt")
                for ch in range(2):
                    pxt = ps_a.tile([128, L, 128], f32, tag="psa")
                    for l in range(L):
                        nc.tensor.matmul(
                            out=pxt[:, l, :P],
                            lhsT=xb[:P, l, ch * 128:(ch + 1) * 128],
                            rhs=ident[:P, :P],
                        )
                    nc.scalar.copy(out=xt[:, ch], in_=pxt)

                # q^T, k^T: [(m,d)ch 128, (l,p) 640] psum -> bf16 sbuf (ACT)
                qkT = work.tile([128, 2, 2, L, 128], bf16, tag="qkT")  # [*, qk, ch, l, p]
                for qk in range(2):
                    for ch in range(2):
                        pqk = ps_a.tile([128, L, 128], f32, tag="psa")
                        for l in range(L):
                            for cc in range(2):
                                nc.tensor.matmul(
                                    out=pqk[:, l, :],
                                    lhsT=wf[:, cc, qk * 256 + ch * 128: qk * 256 + (ch + 1) * 128],
                                    rhs=xt[:, cc, l, :],
                                    start=(cc == 0),
                                    stop=(cc == 1),
                                )
                        nc.scalar.copy(out=qkT[:, qk, ch], in_=pqk)

                # v in p-layout [P, (l, d, m)] bf16 (ACT/DVE copies)
                v_sb = work.tile([128, L, DIM_HEAD, HEADS], bf16, tag="v_sb")
                for gi, (lg0, lg1) in enumerate(((0, 2), (2, 4), (4, 5))):
                    pv = ps_b.tile([128, 2, C], f32, tag="psb")
                    for l in range(lg0, lg1):
                        for cc in range(2):
                            nc.tensor.matmul(
                                out=pv[:P, l - lg0, :],
                                lhsT=xt[:, cc, l, :P],
                                rhs=wf[:, cc, 512:768],
                                start=(cc == 0),
                                stop=(cc == 1),
                            )
                    nc.scalar.copy(
                        out=v_sb[:P, lg0:lg1],
                        in_=pv[:P, 0:lg1 - lg0].rearrange("p l (d m) -> p l d m", m=HEADS),
                    )
                st.update(xt=xt, qkT=qkT, v_sb=v_sb)

            def stage_qkA(t):
                """Mask row, QK products, band-reduce logits (ends tile t's PE work)."""
                st = state[t]
                P, qkT, maskiT = st["P"], st["qkT"], st["maskiT"]

                # mask row -> [L, P] bf16: NBIG*mask - NBIG (DVE converts the i32)
                mrowT = work.tile([L, 128], bf16, tag="mrowT")
                nc.vector.tensor_scalar(
                    out=mrowT[:, :P], in0=maskiT[:, :P],
                    scalar1=NBIG, scalar2=-NBIG,
                    op0=mybir.AluOpType.mult, op1=mybir.AluOpType.add,
                )

                # QK products E[(m,d)ch, i, j, p] bf16 (DVE)
                E = work.tile([128, 2, L, L, 128], bf16, tag="E", bufs=2)
                for ch in range(2):
                    nc.vector.tensor_mul(
                        E[:, ch, :, :, :P],
                        qkT[:, 0, ch, :, :P].unsqueeze(2).broadcast_to([128, L, L, P]),
                        qkT[:, 1, ch, :, :P].unsqueeze(1).broadcast_to([128, L, L, P]),
                    )

                # A[p, (i,j,m)] psum: mask seeds the bank (start=True resets
                # whole psum granules, so it must be the only start), then the
                # band-reduce matmuls accumulate into their 4-col regions.
                pa = ps_pa.tile([128, L, L, HEADS], f32, tag="pspa")
                nc.tensor.matmul(
                    out=pa[:P],
                    lhsT=mrowT[:, :P],
                    rhs=mj,
                    start=True,
                    stop=False,
                    skip_group_check=True,
                )
                for ch in range(2):
                    for i in range(L):
                        for j in range(L):
                            nc.tensor.matmul(
                                out=pa[:P, i, j, ch * 4:(ch + 1) * 4],
                                lhsT=E[:, ch, i, j, :P],
                                rhs=h4,
                                start=False,
                                stop=True,
                                skip_group_check=True,
                            )
                st["pa"] = pa

            def stage_soft(t):
                """exp, softmax, AV products for tile t (logits ready last iter)."""
                st = state[t]
                P, v_sb, pa = st["P"], st["v_sb"], st["pa"]

                # softmax over j (no max subtraction; logits*SCALE in exp)
                ee = work.tile([128, L, L, HEADS], bf16, tag="ee")
                nc.scalar.activation(
                    out=ee[:P], in_=pa[:P], func=mybir.ActivationFunctionType.Exp,
                    scale=SCALE,
                )
                ssum = work.tile([128, L, HEADS], f32, tag="ssum")
                nc.vector.reduce_sum(
                    out=ssum[:P], in_=ee[:P].transpose([0, 1, 3, 2]),
                    axis=mybir.AxisListType.X,
                )
                sinv = work.tile([128, L, HEADS], bf16, tag="sinv")
                with nc.allow_low_precision(reason="bf16 softmax; 2e-2 rel-err budget"):
                    nc.vector.reciprocal(out=sinv[:P], in_=ssum[:P])
                pw = work.tile([128, L, L, HEADS], bf16, tag="pw")
                nc.vector.tensor_mul(
                    pw[:P], ee[:P],
                    sinv[:P].unsqueeze(2).broadcast_to([P, L, L, HEADS]),
                )

                # AV products av[p, (i, j, d, m)] bf16 (DVE)
                av = work.tile([128, L, L, DIM_HEAD, HEADS], bf16, tag="av", bufs=4)
                nc.vector.tensor_mul(
                    av[:P],
                    pw[:P].unsqueeze(3).broadcast_to([P, L, L, DIM_HEAD, HEADS]),
                    v_sb[:P].unsqueeze(1).broadcast_to([P, L, L, DIM_HEAD, HEADS]),
                )
                # pre-sum j0+j1 on gpsimd to shrink the PE transpose-accumulate
                s01 = work.tile([128, L, DIM_HEAD, HEADS], bf16, tag="s01", bufs=4)
                nc.vector.tensor_add(s01[:P], av[:P, :, 0], av[:P, :, 1])
                st["av"] = av
                st["s01"] = s01

            def stage_jsum(t):
                """j-sum transposes accumulating attout^T for tile t."""
                st = state[t]
                P, av, s01 = st["P"], st["av"], st["s01"]

                aot = work.tile([128, 2, L, 128], bf16, tag="aot", bufs=4)
                for ch in range(2):
                    pat = ps_a.tile([128, L, 128], f32, tag="psa")
                    for i in range(L):
                        terms = [s01[:P, i], av[:P, i, 2], av[:P, i, 3], av[:P, i, 4]]
                        for n, term in enumerate(terms):
                            nc.tensor.matmul(
                                out=pat[:, i, :P],
                                lhsT=term.rearrange("p d m -> p (d m)")[
                                    :, ch * 128:(ch + 1) * 128],
                                rhs=ident[:P, :P],
                                start=(n == 0),
                                stop=(n == len(terms) - 1),
                            )
                    nc.scalar.copy(out=aot[:, ch], in_=pat)
                st["aot"] = aot

            def stage_out(t):
                """Output projection, bias, store for tile t."""
                st = state.pop(t)
                P, pieces, aot = st["P"], st["pieces"], st["aot"]

                osb = work.tile([128, L, C], f32, tag="osb", bufs=2)
                for i in range(L):
                    po = ps_b.tile([128, C], f32, tag="psb")
                    nc.tensor.matmul(
                        out=po[:P], lhsT=ones1[:, :P], rhs=bconst,
                        start=True, stop=False, skip_group_check=True,
                    )
                    for ch in range(2):
                        nc.tensor.matmul(
                            out=po[:P],
                            lhsT=aot[:, ch, i, :P],
                            rhs=wf[:, ch, 768:1024],
                            start=False,
                            stop=(ch == 1),
                            skip_group_check=True,
                        )
                    if i in (1, 3):
                        nc.vector.tensor_copy(out=osb[:P, i], in_=po[:P])
                    else:
                        nc.scalar.copy(out=osb[:P, i], in_=po[:P])
                for (p0, b, h, w0, w1) in pieces:
                    nc.sync.dma_start(
                        out=out_d[b, :, h, w0:w1, :].transpose([1, 0, 2]),
                        in_=osb[p0:p0 + (w1 - w0)],
                    )

            # software-pipelined tile loop:
            #  load k+1 | softmax/AV k-1 | proj+QK k | jsum k-2 | outproj k-3
            stage_load(0)
            for k in range(NTILES + 3):
                if k + 1 < NTILES:
                    stage_load(k + 1)
                if k < NTILES:
                    stage_proj(k)
                if 1 <= k <= NTILES:
                    stage_soft(k - 1)
                if 2 <= k <= NTILES + 1:
                    stage_jsum(k - 2)
                if 3 <= k:
                    stage_out(k - 3)
                if k < NTILES:
                    stage_qkA(k)
    nc.finalize()
    return nc


def get_nc():
    if "nc" not in _cached:
        _cached["nc"] = _build_bass()
    return _cached["nc"]


def _in_maps(x, mask, w_qkv, w_out, b_out):
    x = np.ascontiguousarray(np.asarray(x), dtype=np.float32)
    mask = np.ascontiguousarray(np.asarray(mask), dtype=np.int32)
    wfused, bconst = _host_weights(
        np.asarray(w_qkv, dtype=np.float32),
        np.asarray(w_out, dtype=np.float32),
        np.asarray(b_out, dtype=np.float32),
    )
    consts = _host_consts()
    maps = []
    for k in range(NCORES):
        h0, h1 = k * HP, (k + 1) * HP
        maps.append({
            "x": np.ascontiguousarray(x[:, :, h0:h1]),
            "mask": np.ascontiguousarray(mask[:, h0:h1]),
            "wfused": wfused,
            "bconst": bconst,
            "consts": consts,
        })
    return maps


def kernel(x, mask, w_qkv, w_out, b_out):
    from concourse.bass_utils import run_bass_kernel_spmd

    nc = get_nc()
    maps = _in_maps(x, mask, w_qkv, w_out, b_out)
    res = run_bass_kernel_spmd(nc, maps, core_ids=list(range(NCORES)))
    out = np.concatenate([r["out"] for r in res.results], axis=2)
    return out


# revision 5
# speedup vs baseline: 1.0197x; 1.0197x over previous
"""CavAttention Trainium2 kernel, v2.

Computation (per spatial location (b,h,w), L=5 "cav" slots, 8 heads x 32 dim):
  qkv = x @ w_qkv ; att = softmax_j(mask * q_i.k_j / sqrt(d)) ; o = att @ v ; out = o @ w_out + b_out

Distribution: shard the H axis (48) across the 8 cores (6 each); weights replicated.

Per-core strategy (bf16 compute, fp32 accumulation in PSUM):
  - locations ride the 128 SBUF partitions for projections/softmax/AV;
    q,k are produced transposed [(m,d), (l,p)] so the 5x5 QK contraction over
    d becomes narrow-output PE matmuls (band matrix sums d within heads).
  - softmax skips the max-subtraction (logits ~ N(0,1): exp is safe);
    the mask enters as -30*(1-mask) added to logits by one K=5 PE matmul
    whose lhsT is the mask row loaded transposed by the DMA.
  - AV products are one big DVE bf16 multiply; the sum over j is folded
    into PE identity-matmuls accumulating in PSUM, which directly yields
    the transposed operand for the output projection.
  - the tile loop is software-pipelined 4 deep so PE never waits in-order
    on a cross-engine producer of the same tile.
  - GPSIMD cannot touch PSUM: it only does SBUF->SBUF work (x cast, part
    of the QK products); all PSUM evacuations are on ACT/DVE.
"""

import numpy as np

B, L, H, W, C = 2, 5, 48, 176, 256
HEADS, DIM_HEAD = 8, 32
INNER = HEADS * DIM_HEAD  # 256
SCALE = DIM_HEAD ** -0.5
NBIG = 176.0  # mask logit offset (scaled by SCALE inside exp: 176/sqrt(32) ~ 31)
NCORES = 8
HP = H // NCORES  # 6 h-planes per core
NBH = B * HP      # 12 (b,h) blocks per core
LOCS = NBH * W    # 2112 locations per core
PTILE = 128
NTILES = (LOCS + PTILE - 1) // PTILE  # 17

_cached = {}


def _pieces(s, e):
    """Split flat loc range [s,e) into (p0, b, h, w0, w1) pieces within (b,h) blocks."""
    out = []
    cur = s
    while cur < e:
        bh = cur // W
        w0 = cur % W
        w1 = min(W, w0 + (e - cur))
        out.append((cur - s, bh // HP, bh % HP, w0, w1))
        cur += w1 - w0
    return out


def _host_consts():
    """Precomputed constant blob DMA'd once per core (bf16)."""
    import ml_dtypes

    bf = ml_dtypes.bfloat16
    # consts [128, 340]: ident | H4 | MJ (j-expansion over (i,j,m))
    consts = np.zeros((128, 340), dtype=np.float32)
    consts[:, :128] = np.eye(128, dtype=np.float32)
    for m in range(4):  # H4: band over d within a 4-head chunk
        consts[m * 32:(m + 1) * 32, 128 + m] = 1.0
    for jp in range(L):  # MJ[j', (i,j,m)] = delta_{j,j'}
        for i in range(L):
            for m in range(HEADS):
                consts[jp, 132 + (i * L + jp) * HEADS + m] = 1.0
    return consts.astype(bf)


def _host_weights(w_qkv, w_out, b_out):
    import ml_dtypes

    bf = ml_dtypes.bfloat16
    wq = w_qkv[:, 0:INNER]                      # [c, (m,d)]
    wk = w_qkv[:, INNER:2 * INNER]              # [c, (m,d)]
    wv = w_qkv[:, 2 * INNER:3 * INNER]          # [c, (m,d)]
    # v is consumed as [p, (l, d, m)]: reorder wv cols (m,d) -> (d,m)
    wv2 = wv.reshape(C, HEADS, DIM_HEAD).transpose(0, 2, 1).reshape(C, INNER)
    # out-proj contracts over (d,m): reorder w_out rows (m,d) -> (d,m)
    wo2 = w_out.reshape(HEADS, DIM_HEAD, C).transpose(1, 0, 2).reshape(INNER, C)
    wfused = np.concatenate([wq, wk, wv2, wo2], axis=1)  # [256, 1024]
    return wfused.astype(bf), b_out.reshape(1, C).astype(bf)


def _build_bass():
    import concourse.bass as bass
    import concourse.bacc as bacc
    import concourse.tile as tile
    from concourse import mybir

    f32 = mybir.dt.float32
    bf16 = mybir.dt.bfloat16
    i32 = mybir.dt.int32

    nc = bacc.Bacc()
    x_d = nc.dram_tensor("x", [B, L, HP, W, C], f32, kind="ExternalInput")
    m_d = nc.dram_tensor("mask", [B, HP, W, 1, L], i32, kind="ExternalInput")
    wf_d = nc.dram_tensor("wfused", [C, 4 * C], bf16, kind="ExternalInput")
    bout_d = nc.dram_tensor("bconst", [1, C], bf16, kind="ExternalInput")
    cst_d = nc.dram_tensor("consts", [128, 340], bf16, kind="ExternalInput")
    out_d = nc.dram_tensor("out", [B, L, HP, W, C], f32, kind="ExternalOutput")

    with tile.TileContext(nc) as tc:
        with (
            tc.tile_pool(name="singles", bufs=1) as singles,
            tc.tile_pool(name="work", bufs=4) as work,
            tc.tile_pool(name="ps_a", bufs=2, space="PSUM") as ps_a,  # 2-bank tiles
            tc.tile_pool(name="ps_b", bufs=2, space="PSUM") as ps_b,  # 1-bank tiles
            tc.tile_pool(name="ps_pa", bufs=1, space="PSUM") as ps_pa,  # logits
        ):
            # ---- one-time constants ----
            cst = singles.tile([128, 340], bf16)
            nc.sync.dma_start(out=cst, in_=cst_d[:, :])
            ident = cst[:, 0:128]
            h4 = cst[:, 128:132]
            mj = cst[0:L, 132:332]
            wf = singles.tile([128, 2, 4 * C], bf16)
            for cc in range(2):
                nc.sync.dma_start(out=wf[:, cc, :], in_=wf_d[cc * 128:(cc + 1) * 128, :])
            bconst = singles.tile([1, C], bf16)
            nc.sync.dma_start(out=bconst, in_=bout_d[:, :])
            ones1 = singles.tile([1, 128], bf16)
            nc.gpsimd.memset(ones1, 1.0)

            state = {}

            def stage_load(t):
                """DMA loads for tile t (prefetched one iteration ahead)."""
                s = t * PTILE
                e = min(s + PTILE, LOCS)
                P = e - s
                pieces = _pieces(s, e)

                x_raw = work.tile([128, L, C], f32, tag="x_raw", bufs=2)
                for (p0, b, h, w0, w1) in pieces:
                    nc.sync.dma_start(
                        out=x_raw[p0:p0 + (w1 - w0), :, :],
                        in_=x_d[b, :, h, w0:w1, :].transpose([1, 0, 2]),
                    )
                # mask loaded transposed: [L(j), p]
                maskiT = work.tile([L, 128], i32, tag="maskiT")
                for (p0, b, h, w0, w1) in pieces:
                    nc.sync.dma_start(
                        out=maskiT[:, p0:p0 + (w1 - w0)],
                        in_=m_d[b, h, w0:w1, 0, :].transpose([1, 0]),
                    )
                xb = work.tile([128, L, C], bf16, tag="xb")
                nc.gpsimd.tensor_copy(out=xb[:P], in_=x_raw[:P])
                state[t] = dict(P=P, pieces=pieces, xb=xb, maskiT=maskiT)

            def stage_proj(t):
                """Cast, transposes, projections for tile t."""
                st = state[t]
                P, xb = st["P"], st["xb"]

                # x^T via PE identity-matmuls: xt [(c)2ch, l, p] bf16
                xt = work.tile([128, 2, L, 128], bf16, tag="xt")
                for ch in range(2):
                    pxt = ps_a.tile([128, L, 128], f32, tag="psa")
                    for l in range(L):
                        nc.tensor.matmul(
                            out=pxt[:, l, :P],
                            lhsT=xb[:P, l, ch * 128:(ch + 1) * 128],
                            rhs=ident[:P, :P],
                        )
                    nc.scalar.copy(out=xt[:, ch], in_=pxt)

                # q^T, k^T: [(m,d)ch 128, (l,p) 640] psum -> bf16 sbuf (ACT)
                qkT = work.tile([128, 2, 2, L, 128], bf16, tag="qkT")  # [*, qk, ch, l, p]
                for ch in range(2):
                    for qk in range(2):
                        pqk = ps_a.tile([128, L, 128], f32, tag="psa")
                        for l in range(L):
                            for cc in range(2):
                                nc.tensor.matmul(
                                    out=pqk[:, l, :],
                                    lhsT=wf[:, cc, qk * 256 + ch * 128: qk * 256 + (ch + 1) * 128],
                                    rhs=xt[:, cc, l, :],
                                    start=(cc == 0),
                                    stop=(cc == 1),
                                )
                        nc.scalar.copy(out=qkT[:, qk, ch], in_=pqk)

                # v in p-layout [P, (l, d, m)] bf16 (ACT/DVE copies)
                v_sb = work.tile([128, L, DIM_HEAD, HEADS], bf16, tag="v_sb")
                for gi, (lg0, lg1) in enumerate(((0, 2), (2, 4), (4, 5))):
                    pv = ps_b.tile([128, 2, C], f32, tag="psb")
                    for l in range(lg0, lg1):
                        for cc in range(2):
                            nc.tensor.matmul(
                                out=pv[:P, l - lg0, :],
                                lhsT=xt[:, cc, l, :P],
                                rhs=wf[:, cc, 512:768],
                                start=(cc == 0),
                                stop=(cc == 1),
                            )
                    nc.scalar.copy(
                        out=v_sb[:P, lg0:lg1],
                        in_=pv[:P, 0:lg1 - lg0].rearrange("p l (d m) -> p l d m", m=HEADS),
                    )
                st.update(xt=xt, qkT=qkT, v_sb=v_sb)

            def stage_qkA(t):
                """Mask row, QK products, band-reduce logits (ends tile t's PE work)."""
                st = state[t]
                P, qkT, maskiT = st["P"], st["qkT"], st["maskiT"]

                # mask row -> [L, P] bf16: NBIG*mask - NBIG (DVE converts the i32)
                mrowT = work.tile([L, 128], bf16, tag="mrowT")
                nc.vector.tensor_scalar(
                    out=mrowT[:, :P], in0=maskiT[:, :P],
                    scalar1=NBIG, scalar2=-NBIG,
                    op0=mybir.AluOpType.mult, op1=mybir.AluOpType.add,
                )

                # QK products E[(m,d)ch, i, j, p] bf16 (DVE)
                E = work.tile([128, 2, L, L, 128], bf16, tag="E", bufs=2)
                for ch in range(2):
                    nc.vector.tensor_mul(
                        E[:, ch, 0:4, :, :P],
                        qkT[:, 0, ch, 0:4, :P].unsqueeze(2).broadcast_to([128, 4, L, P]),
                        qkT[:, 1, ch, :, :P].unsqueeze(1).broadcast_to([128, 4, L, P]),
                    )
                    nc.gpsimd.tensor_mul(
                        E[:, ch, 4, :, :P],
                        qkT[:, 0, ch, 4, :P].unsqueeze(1).broadcast_to([128, L, P]),
                        qkT[:, 1, ch, :, :P],
                    )

                # A[p, (i,j,m)] psum: mask seeds the bank (start=True resets
                # whole psum granules, so it must be the only start), then the
                # band-reduce matmuls accumulate into their 4-col regions.
                pa = ps_pa.tile([128, L, L, HEADS], f32, tag="pspa")
                nc.tensor.matmul(
                    out=pa[:P],
                    lhsT=mrowT[:, :P],
                    rhs=mj,
                    start=True,
                    stop=False,
                    skip_group_check=True,
                )
                for ch in range(2):
                    for i in range(L):
                        for j in range(L):
                            nc.tensor.matmul(
                                out=pa[:P, i, j, ch * 4:(ch + 1) * 4],
                                lhsT=E[:, ch, i, j, :P],
                                rhs=h4,
                                start=False,
                                stop=True,
                                skip_group_check=True,
                            )
                st["pa"] = pa

            def stage_soft(t):
                """exp, softmax, AV products for tile t (logits ready last iter)."""
                st = state[t]
                P, v_sb, pa = st["P"], st["v_sb"], st["pa"]

                # softmax over j (no max subtraction; logits*SCALE in exp)
                ee = work.tile([128, L, L, HEADS], bf16, tag="ee")
                nc.scalar.activation(
                    out=ee[:P], in_=pa[:P], func=mybir.ActivationFunctionType.Exp,
                    scale=SCALE,
                )
                ssum = work.tile([128, L, HEADS], f32, tag="ssum")
                nc.vector.reduce_sum(
                    out=ssum[:P], in_=ee[:P].transpose([0, 1, 3, 2]),
                    axis=mybir.AxisListType.X,
                )
                sinv = work.tile([128, L, HEADS], bf16, tag="sinv")
                with nc.allow_low_precision(reason="bf16 softmax; 2e-2 rel-err budget"):
                    nc.vector.reciprocal(out=sinv[:P], in_=ssum[:P])
                pw = work.tile([128, L, L, HEADS], bf16, tag="pw")
                nc.vector.tensor_mul(
                    pw[:P], ee[:P],
                    sinv[:P].unsqueeze(2).broadcast_to([P, L, L, HEADS]),
                )

                # AV products av[p, (i, j, d, m)] bf16 (DVE)
                av = work.tile([128, L, L, DIM_HEAD, HEADS], bf16, tag="av", bufs=3)
                nc.vector.tensor_mul(
                    av[:P, 0:4],
                    pw[:P, 0:4].unsqueeze(3).broadcast_to([P, 4, L, DIM_HEAD, HEADS]),
                    v_sb[:P].unsqueeze(1).broadcast_to([P, 4, L, DIM_HEAD, HEADS]),
                )
                nc.gpsimd.tensor_mul(
                    av[:P, 4],
                    pw[:P, 4].unsqueeze(2).broadcast_to([P, L, DIM_HEAD, HEADS]),
                    v_sb[:P],
                )
                # pre-sum j0+j1 on gpsimd to shrink the PE transpose-accumulate
                s01 = work.tile([128, L, DIM_HEAD, HEADS], bf16, tag="s01", bufs=4)
                nc.vector.tensor_add(s01[:P], av[:P, :, 0], av[:P, :, 1])
                st["av"] = av
                st["s01"] = s01

            def stage_jsum(t):
                """j-sum transposes accumulating attout^T for tile t."""
                st = state[t]
                P, av, s01 = st["P"], st["av"], st["s01"]

                aot = work.tile([128, 2, L, 128], bf16, tag="aot", bufs=4)
                for ch in range(2):
                    pat = ps_a.tile([128, L, 128], f32, tag="psa")
                    for i in range(L):
                        terms = [s01[:P, i], av[:P, i, 2], av[:P, i, 3], av[:P, i, 4]]
                        for n, term in enumerate(terms):
                            nc.tensor.matmul(
                                out=pat[:, i, :P],
                                lhsT=term.rearrange("p d m -> p (d m)")[
                                    :, ch * 128:(ch + 1) * 128],
                                rhs=ident[:P, :P],
                                start=(n == 0),
                                stop=(n == len(terms) - 1),
                            )
                    nc.scalar.copy(out=aot[:, ch], in_=pat)
                st["aot"] = aot

            def stage_out(t):
                """Output projection, bias, store for tile t."""
                st = state.pop(t)
                P, pieces, aot = st["P"], st["pieces"], st["aot"]

                osb = work.tile([128, L, C], f32, tag="osb", bufs=2)
                for i in range(L):
                    po = ps_b.tile([128, C], f32, tag="psb")
                    nc.tensor.matmul(
                        out=po[:P], lhsT=ones1[:, :P], rhs=bconst,
                        start=True, stop=False, skip_group_check=True,
                    )
                    for ch in range(2):
                        nc.tensor.matmul(
                            out=po[:P],
                            lhsT=aot[:, ch, i, :P],
                            rhs=wf[:, ch, 768:1024],
                            start=False,
                            stop=(ch == 1),
                            skip_group_check=True,
                        )
                    if i in (1, 3, 4):
                        nc.vector.tensor_copy(out=osb[:P, i], in_=po[:P])
                    else:
                        nc.scalar.copy(out=osb[:P, i], in_=po[:P])
                for (p0, b, h, w0, w1) in pieces:
                    nc.sync.dma_start(
                        out=out_d[b, :, h, w0:w1, :].transpose([1, 0, 2]),
                        in_=osb[p0:p0 + (w1 - w0)],
                    )

            # software-pipelined tile loop:
            #  load k+1 | softmax/AV k-1 | proj+QK k | jsum k-2 | outproj k-3
            stage_load(0)
            for k in range(NTILES + 3):
                if k + 1 < NTILES:
                    stage_load(k + 1)
                if k < NTILES:
                    stage_proj(k)
                if 1 <= k <= NTILES:
                    stage_soft(k - 1)
                if 2 <= k <= NTILES + 1:
                    stage_jsum(k - 2)
                if 3 <= k:
                    stage_out(k - 3)
                if k < NTILES:
                    stage_qkA(k)
    nc.finalize()
    return nc


def get_nc():
    if "nc" not in _cached:
        _cached["nc"] = _build_bass()
    return _cached["nc"]


def _in_maps(x, mask, w_qkv, w_out, b_out):
    x = np.ascontiguousarray(np.asarray(x), dtype=np.float32)
    mask = np.ascontiguousarray(np.asarray(mask), dtype=np.int32)
    wfused, bconst = _host_weights(
        np.asarray(w_qkv, dtype=np.float32),
        np.asarray(w_out, dtype=np.float32),
        np.asarray(b_out, dtype=np.float32),
    )
    consts = _host_consts()
    maps = []
    for k in range(NCORES):
        h0, h1 = k * HP, (k + 1) * HP
        maps.append({
            "x": np.ascontiguousarray(x[:, :, h0:h1]),
            "mask": np.ascontiguousarray(mask[:, h0:h1]),
            "wfused": wfused,
            "bconst": bconst,
            "consts": consts,
        })
    return maps


def kernel(x, mask, w_qkv, w_out, b_out):
    from concourse.bass_utils import run_bass_kernel_spmd

    nc = get_nc()
    maps = _in_maps(x, mask, w_qkv, w_out, b_out)
    res = run_bass_kernel_spmd(nc, maps, core_ids=list(range(NCORES)))
    out = np.concatenate([r["out"] for r in res.results], axis=2)
    return out
